# revision 7
# baseline (speedup 1.0000x reference)
import sys

sys.path.insert(0, "/opt/trn_rl_repo")

import numpy as np
import ml_dtypes

import concourse.bass as bass
import concourse.tile as tile
from concourse import mybir
from concourse import bass_utils
from concourse import bacc

B, T, S = 4, 2048, 2048
DQ, DKV, H, HD = 512, 1024, 8, 64
N_CORES = 8
SC = S // 2  # s-rows handled per core (batch b = c//2, s-half = c%2)

BF16 = ml_dtypes.bfloat16

_prog_cache = {}


def _build_program():
    f32 = mybir.dt.float32
    bf16 = mybir.dt.bfloat16
    FP = mybir.ActivationFunctionType

    nc = bacc.Bacc("TRN2", target_bir_lowering=False, debug=False,
                   num_devices=N_CORES)

    qT = nc.dram_tensor("qT", [DQ, T], bf16, kind="ExternalInput").ap()
    kT = nc.dram_tensor("kT", [DKV, SC], bf16, kind="ExternalInput").ap()
    vT = nc.dram_tensor("vT", [DKV, SC], bf16, kind="ExternalInput").ap()
    wq = nc.dram_tensor("wq", [DQ, DQ], bf16, kind="ExternalInput").ap()
    wk = nc.dram_tensor("wk", [DKV, DQ], bf16, kind="ExternalInput").ap()
    wv = nc.dram_tensor("wv", [DKV, DQ], bf16, kind="ExternalInput").ap()
    # unnormalized exp(scores)^T per head: [h, s_local, t] bf16
    attn_e = nc.dram_tensor("attn_e", [H, SC, T], bf16,
                            kind="ExternalOutput").ap()
    # PV partials + denominator row: [h, 65, t] f32
    pv = nc.dram_tensor("pv", [H, HD + 1, T], f32, kind="ExternalOutput").ap()

    with tile.TileContext(nc) as tc:
        with (
            tc.tile_pool(name="wpool", bufs=1) as wpool,
            tc.tile_pool(name="persist", bufs=1) as ppool,
            tc.tile_pool(name="psum", bufs=2, space="PSUM") as pspool,
            tc.tile_pool(name="psum_pv", bufs=1, space="PSUM") as pvpool,
        ):
            # ---- load weights ----
            wq_sb = wpool.tile([128, 4, DQ], bf16, tag="wq")
            nc.sync.dma_start(out=wq_sb[:], in_=wq.rearrange("(a p) n -> p a n", p=128))
            wk_sb = wpool.tile([128, 8, DQ], bf16, tag="wk")
            nc.sync.dma_start(out=wk_sb[:], in_=wk.rearrange("(a p) n -> p a n", p=128))
            wv_sb = wpool.tile([128, 8, DQ], bf16, tag="wv")
            nc.sync.dma_start(out=wv_sb[:], in_=wv.rearrange("(a p) n -> p a n", p=128))

            # ---- persistent intermediates ----
            QT_sb = ppool.tile([128, 4, T], bf16, tag="qt")      # Q^T [dq, t]
            KT_sb = ppool.tile([128, 4, SC], bf16, tag="kt")     # K^T [dq, s_local]
            V_sb = ppool.tile([128, 8, H, HD + 1], bf16, tag="v")  # V [s,h,hd+1]

            nc.vector.memset(V_sb[:, :, :, HD:HD + 1], 1.0)

            # ---- P1: projections ----
            with tc.tile_pool(name="big", bufs=1) as bigpool:
                qT_view = qT.rearrange("(a p) n -> p a n", p=128)
                qt0 = bigpool.tile([128, 2, T], bf16, tag="qt0")
                nc.sync.dma_start(out=qt0[:], in_=qT_view[:, 0:2, :])
                qt1 = bigpool.tile([128, 2, T], bf16, tag="qt1")
                nc.sync.dma_start(out=qt1[:], in_=qT_view[:, 2:4, :])
                kT_view = kT.rearrange("(a p) n -> p a n", p=128)
                kt0 = bigpool.tile([128, 4, SC], bf16, tag="kt0")
                nc.sync.dma_start(out=kt0[:], in_=kT_view[:, 0:4, :])
                kt1 = bigpool.tile([128, 4, SC], bf16, tag="kt1")
                nc.sync.dma_start(out=kt1[:], in_=kT_view[:, 4:8, :])
                vT_view = vT.rearrange("(a p) n -> p a n", p=128)
                vt0 = bigpool.tile([128, 4, SC], bf16, tag="vt0")
                nc.sync.dma_start(out=vt0[:], in_=vT_view[:, 0:4, :])
                vt1 = bigpool.tile([128, 4, SC], bf16, tag="vt1")
                nc.sync.dma_start(out=vt1[:], in_=vT_view[:, 4:8, :])

                # QT = Wq'^T @ qT  (Wq' pre-scaled by 1/sqrt(HD) on host)
                for m in range(4):
                    for half in range(2):
                        ps = pspool.tile([128, 1024], f32, tag="ps")
                        for nh in range(2):
                            nq = half * 2 + nh
                            for k in range(4):
                                qt = qt0 if k < 2 else qt1
                                nc.tensor.matmul(
                                    ps[:, nh * 512:(nh + 1) * 512],
                                    lhsT=wq_sb[:, k, m * 128:(m + 1) * 128],
                                    rhs=qt[:, k % 2, nq * 512:(nq + 1) * 512],
                                    start=(k == 0), stop=(k == 3),
                                )
                        nc.scalar.activation(
                            QT_sb[:, m, half * 1024:(half + 1) * 1024],
                            ps[:], FP.Copy)

                # KT = Wk^T @ kT
                for m in range(4):
                    ps = pspool.tile([128, 1024], f32, tag="ps")
                    for nh in range(2):
                        for k in range(8):
                            kt = kt0 if k < 4 else kt1
                            nc.tensor.matmul(
                                ps[:, nh * 512:(nh + 1) * 512],
                                lhsT=wk_sb[:, k, m * 128:(m + 1) * 128],
                                rhs=kt[:, k % 4, nh * 512:(nh + 1) * 512],
                                start=(k == 0), stop=(k == 7),
                            )
                    nc.scalar.activation(KT_sb[:, m, :], ps[:], FP.Copy)

                # V = value @ Wv, stored [s_block, h, hd] with ones col
                for st in range(8):
                    ps = pspool.tile([128, DQ], f32, tag="ps")
                    for k in range(8):
                        vt = vt0 if k < 4 else vt1
                        nc.tensor.matmul(
                            ps[:, 0:DQ],
                            lhsT=vt[:, k % 4, st * 128:(st + 1) * 128],
                            rhs=wv_sb[:, k, :],
                            start=(k == 0), stop=(k == 7),
                        )
                    nc.scalar.activation(
                        V_sb[:, st, :, 0:HD],
                        ps[:, 0:DQ].rearrange("p (h d) -> p h d", d=HD),
                        FP.Copy,
                    )

            # ---- P3: attention per head (unnormalized) ----
            with (
                tc.tile_pool(name="a_u", bufs=6) as apool,
                tc.tile_pool(name="pv_sb", bufs=2) as pvsb_pool,
            ):
                for h in range(8):
                    ch, po = h // 2, (h % 2) * 64
                    ps_pv = pvpool.tile([HD + 1, T], f32, tag="pv")
                    for st in range(8):
                        for tq in range(2):
                            ps_sc = pspool.tile([128, 1024], f32, tag="ps")
                            for nh in range(2):
                                nc.tensor.matmul(
                                    ps_sc[:, nh * 512:(nh + 1) * 512],
                                    lhsT=KT_sb[po:po + 64, ch,
                                               st * 128:(st + 1) * 128],
                                    rhs=QT_sb[po:po + 64, ch,
                                              tq * 1024 + nh * 512:
                                              tq * 1024 + (nh + 1) * 512],
                                    start=True, stop=True,
                                )
                            a = apool.tile([128, 1024], bf16)
                            nc.scalar.activation(a[:], ps_sc[:], FP.Exp)
                            for nh in range(2):
                                nc.tensor.matmul(
                                    ps_pv[:, tq * 1024 + nh * 512:
                                          tq * 1024 + (nh + 1) * 512],
                                    lhsT=V_sb[:, st, h, :],
                                    rhs=a[:, nh * 512:(nh + 1) * 512],
                                    start=(st == 0), stop=(st == 7),
                                )
                            nc.sync.dma_start(
                                out=attn_e[h, st * 128:(st + 1) * 128,
                                           tq * 1024:(tq + 1) * 1024],
                                in_=a[:])
                    pv_sb = pvsb_pool.tile([HD + 1, T], f32, tag="pvsb")
                    nc.scalar.activation(pv_sb[:], ps_pv[:], FP.Copy)
                    nc.sync.dma_start(out=pv[h], in_=pv_sb[:])

    nc.compile()
    return nc


def _fallback(query, key, value, key_padding_mask, attn_mask,
              Wq, bq, Wk, bk, Wv, bv, Wo, bo):
    scale = np.float32(np.sqrt(HD))
    Q = (query @ Wq + bq).reshape(B, T, H, HD)
    K = (key @ Wk + bk).reshape(B, S, H, HD)
    V = (value @ Wv + bv).reshape(B, S, H, HD)
    scores = np.einsum("bthd,bshd->bhts", Q, K) / scale
    scores = scores + attn_mask[None, None, :, :]
    pad = (key_padding_mask == 0)[:, None, None, :]
    scores = np.where(pad, -np.inf, scores)
    scores = scores - scores.max(axis=-1, keepdims=True)
    e = np.exp(scores)
    attn = e / e.sum(axis=-1, keepdims=True)
    out = np.einsum("bhts,bshd->bthd", attn, V).reshape(B, T, DQ)
    out = out @ Wo + bo
    return out.astype(np.float32), attn.astype(np.float32)


def kernel(query, key, value, key_padding_mask, attn_mask,
           Wq, bq, Wk, bk, Wv, bv, Wo, bo):
    query = np.asarray(query, np.float32)
    key = np.asarray(key, np.float32)
    value = np.asarray(value, np.float32)
    key_padding_mask = np.asarray(key_padding_mask)
    attn_mask = np.asarray(attn_mask, np.float32)
    Wq, Wk, Wv, Wo = (np.asarray(w, np.float32) for w in (Wq, Wk, Wv, Wo))
    bq, bk, bv, bo = (np.asarray(b, np.float32) for b in (bq, bk, bv, bo))

    fast = (
        not attn_mask.any()
        and np.all(key_padding_mask != 0)
        and not bq.any() and not bk.any() and not bv.any() and not bo.any()
    )
    if not fast:
        return _fallback(query, key, value, key_padding_mask, attn_mask,
                         Wq, bq, Wk, bk, Wv, bv, Wo, bo)

    if "nc" not in _prog_cache:
        _prog_cache["nc"] = _build_program()
    nc = _prog_cache["nc"]

    wq_b = (Wq / np.float32(np.sqrt(HD))).astype(BF16)
    wk_b = Wk.astype(BF16)
    wv_b = Wv.astype(BF16)

    in_maps = []
    for c in range(N_CORES):
        b, sh = c // 2, c % 2
        in_maps.append({
            "qT": np.ascontiguousarray(query[b].T).astype(BF16),
            "kT": np.ascontiguousarray(
                key[b, sh * SC:(sh + 1) * SC, :].T).astype(BF16),
            "vT": np.ascontiguousarray(
                value[b, sh * SC:(sh + 1) * SC, :].T).astype(BF16),
            "wq": wq_b, "wk": wk_b, "wv": wv_b,
        })

    res = bass_utils.run_bass_kernel_spmd(nc, in_maps, core_ids=list(range(N_CORES)))

    out_full = np.empty((B, T, DQ), np.float32)
    attn_full = np.empty((B, H, T, S), np.float32)
    for b in range(B):
        p0 = res.results[2 * b]["pv"]          # [H, 65, T] f32
        p1 = res.results[2 * b + 1]["pv"]
        inv = 1.0 / (p0[:, HD, :] + p1[:, HD, :])          # [H, T]
        for sh in range(2):
            e = res.results[2 * b + sh]["attn_e"]          # [H, SC, T] bf16
            attn_full[b, :, :, sh * SC:(sh + 1) * SC] = (
                e.astype(np.float32) * inv[:, None, :]).transpose(0, 2, 1)
        ctx = (p0[:, :HD, :] + p1[:, :HD, :]) * inv[:, None, :]  # [H, 64, T]
        out_full[b] = (
            ctx.transpose(2, 0, 1).reshape(T, DQ) @ Wo)
    return out_full, attn_full


# revision 9
# speedup vs baseline: 1.2277x; 1.2277x over previous
import sys

sys.path.insert(0, "/opt/trn_rl_repo")

import numpy as np
import ml_dtypes

import concourse.bass as bass
import concourse.tile as tile
from concourse import mybir
from concourse import bass_utils
from concourse import bacc

B, T, S = 4, 2048, 2048
DQ, DKV, H, HD = 512, 1024, 8, 64
N_CORES = 8
SC = S // 2  # s-rows handled per core (batch b = c//2, s-half = c%2)

BF16 = ml_dtypes.bfloat16

_prog_cache = {}


def _build_program():
    f32 = mybir.dt.float32
    bf16 = mybir.dt.bfloat16
    FP = mybir.ActivationFunctionType

    nc = bacc.Bacc("TRN2", target_bir_lowering=False, debug=False,
                   num_devices=N_CORES)

    qT = nc.dram_tensor("qT", [DQ, T], bf16, kind="ExternalInput").ap()
    kT = nc.dram_tensor("kT", [DKV, SC], bf16, kind="ExternalInput").ap()
    vT = nc.dram_tensor("vT", [DKV, SC], bf16, kind="ExternalInput").ap()
    wq = nc.dram_tensor("wq", [DQ, DQ], bf16, kind="ExternalInput").ap()
    wk = nc.dram_tensor("wk", [DKV, DQ], bf16, kind="ExternalInput").ap()
    wv = nc.dram_tensor("wv", [DKV, DQ], bf16, kind="ExternalInput").ap()
    # unnormalized exp(scores)^T per head: [h, s_local, t] bf16
    attn_e = nc.dram_tensor("attn_e", [H, SC, T], bf16,
                            kind="ExternalOutput").ap()
    # PV partials + denominator row: [h, 65, t] f32
    pv = nc.dram_tensor("pv", [H, HD + 1, T], f32, kind="ExternalOutput").ap()

    with tile.TileContext(nc) as tc:
        with (
            tc.tile_pool(name="wpool", bufs=1) as wpool,
            tc.tile_pool(name="persist", bufs=1) as ppool,
            tc.tile_pool(name="psum", bufs=2, space="PSUM") as pspool,
            tc.tile_pool(name="psum_pv", bufs=1, space="PSUM") as pvpool,
        ):
            # ---- load weights (wq first so Q proj starts ASAP) ----
            wq_sb = wpool.tile([128, 4, DQ], bf16, tag="wq")
            nc.sync.dma_start(out=wq_sb[:], in_=wq.rearrange("(a p) n -> p a n", p=128))
            wk_sb = wpool.tile([128, 8, DQ], bf16, tag="wk")
            wv_sb = wpool.tile([128, 8, DQ], bf16, tag="wv")

            # ---- persistent intermediates ----
            QT_sb = ppool.tile([128, 4, T], bf16, tag="qt")      # Q^T [dq, t]
            KT_sb = ppool.tile([128, 4, SC], bf16, tag="kt")     # K^T [dq, s_local]
            V_sb = ppool.tile([128, 8, H, HD + 1], bf16, tag="v")  # V [s,h,hd+1]

            nc.vector.memset(V_sb[:, :, :, HD:HD + 1], 1.0)

            # ---- P1: projections ----
            with tc.tile_pool(name="big", bufs=1) as bigpool:
                qT_view = qT.rearrange("(a p) n -> p a n", p=128)
                qt0 = bigpool.tile([128, 2, T], bf16, tag="qt0")
                nc.sync.dma_start(out=qt0[:], in_=qT_view[:, 0:2, :])
                qt1 = bigpool.tile([128, 2, T], bf16, tag="qt1")
                nc.sync.dma_start(out=qt1[:], in_=qT_view[:, 2:4, :])
                kT_view = kT.rearrange("(a p) n -> p a n", p=128)
                kt0 = bigpool.tile([128, 4, SC], bf16, tag="kt0")
                nc.sync.dma_start(out=kt0[:], in_=kT_view[:, 0:4, :])
                kt1 = bigpool.tile([128, 4, SC], bf16, tag="kt1")
                nc.sync.dma_start(out=kt1[:], in_=kT_view[:, 4:8, :])
                nc.sync.dma_start(out=wk_sb[:], in_=wk.rearrange("(a p) n -> p a n", p=128))
                vT_view = vT.rearrange("(a p) n -> p a n", p=128)
                vt0 = bigpool.tile([128, 4, SC], bf16, tag="vt0")
                nc.sync.dma_start(out=vt0[:], in_=vT_view[:, 0:4, :])
                vt1 = bigpool.tile([128, 4, SC], bf16, tag="vt1")
                nc.sync.dma_start(out=vt1[:], in_=vT_view[:, 4:8, :])
                nc.sync.dma_start(out=wv_sb[:], in_=wv.rearrange("(a p) n -> p a n", p=128))

                # QT = Wq'^T @ qT  (Wq' pre-scaled by 1/sqrt(HD) on host)
                for m in range(4):
                    for half in range(2):
                        ps = pspool.tile([128, 1024], f32, tag="ps")
                        for nh in range(2):
                            nq = half * 2 + nh
                            for k in range(4):
                                qt = qt0 if k < 2 else qt1
                                nc.tensor.matmul(
                                    ps[:, nh * 512:(nh + 1) * 512],
                                    lhsT=wq_sb[:, k, m * 128:(m + 1) * 128],
                                    rhs=qt[:, k % 2, nq * 512:(nq + 1) * 512],
                                    start=(k == 0), stop=(k == 3),
                                )
                        nc.scalar.activation(
                            QT_sb[:, m, half * 1024:(half + 1) * 1024],
                            ps[:], FP.Copy)

                # KT = Wk^T @ kT
                for m in range(4):
                    ps = pspool.tile([128, 1024], f32, tag="ps")
                    for nh in range(2):
                        for k in range(8):
                            kt = kt0 if k < 4 else kt1
                            nc.tensor.matmul(
                                ps[:, nh * 512:(nh + 1) * 512],
                                lhsT=wk_sb[:, k, m * 128:(m + 1) * 128],
                                rhs=kt[:, k % 4, nh * 512:(nh + 1) * 512],
                                start=(k == 0), stop=(k == 7),
                            )
                    nc.scalar.activation(KT_sb[:, m, :], ps[:], FP.Copy)

                # V = value @ Wv, stored [s_block, h, hd] with ones col
                for st in range(8):
                    ps = pspool.tile([128, DQ], f32, tag="ps")
                    for k in range(8):
                        vt = vt0 if k < 4 else vt1
                        nc.tensor.matmul(
                            ps[:, 0:DQ],
                            lhsT=vt[:, k % 4, st * 128:(st + 1) * 128],
                            rhs=wv_sb[:, k, :],
                            start=(k == 0), stop=(k == 7),
                        )
                    nc.scalar.activation(
                        V_sb[:, st, :, 0:HD],
                        ps[:, 0:DQ].rearrange("p (h d) -> p h d", d=HD),
                        FP.Copy,
                    )

            # ---- P3: attention per head (unnormalized) ----
            with (
                tc.tile_pool(name="a_u", bufs=6) as apool,
                tc.tile_pool(name="pv_sb", bufs=2) as pvsb_pool,
            ):
                for h in range(8):
                    ch, po = h // 2, (h % 2) * 64
                    ps_pv = pvpool.tile([HD + 1, T], f32, tag="pv")
                    for st in range(8):
                        for tq in range(2):
                            ps_sc = pspool.tile([128, 1024], f32, tag="ps")
                            for nh in range(2):
                                nc.tensor.matmul(
                                    ps_sc[:, nh * 512:(nh + 1) * 512],
                                    lhsT=KT_sb[po:po + 64, ch,
                                               st * 128:(st + 1) * 128],
                                    rhs=QT_sb[po:po + 64, ch,
                                              tq * 1024 + nh * 512:
                                              tq * 1024 + (nh + 1) * 512],
                                    start=True, stop=True,
                                )
                            a = apool.tile([128, 1024], bf16)
                            nc.scalar.activation(a[:], ps_sc[:], FP.Exp)
                            for nh in range(2):
                                nc.tensor.matmul(
                                    ps_pv[:, tq * 1024 + nh * 512:
                                          tq * 1024 + (nh + 1) * 512],
                                    lhsT=V_sb[:, st, h, :],
                                    rhs=a[:, nh * 512:(nh + 1) * 512],
                                    start=(st == 0), stop=(st == 7),
                                )
                            nc.sync.dma_start(
                                out=attn_e[h, st * 128:(st + 1) * 128,
                                           tq * 1024:(tq + 1) * 1024],
                                in_=a[:])
                    pv_sb = pvsb_pool.tile([HD + 1, T], f32, tag="pvsb")
                    nc.scalar.activation(pv_sb[:], ps_pv[:], FP.Copy)
                    nc.sync.dma_start(out=pv[h], in_=pv_sb[:])

    nc.compile()
    return nc


def _fallback(query, key, value, key_padding_mask, attn_mask,
              Wq, bq, Wk, bk, Wv, bv, Wo, bo):
    scale = np.float32(np.sqrt(HD))
    Q = (query @ Wq + bq).reshape(B, T, H, HD)
    K = (key @ Wk + bk).reshape(B, S, H, HD)
    V = (value @ Wv + bv).reshape(B, S, H, HD)
    scores = np.einsum("bthd,bshd->bhts", Q, K) / scale
    scores = scores + attn_mask[None, None, :, :]
    pad = (key_padding_mask == 0)[:, None, None, :]
    scores = np.where(pad, -np.inf, scores)
    scores = scores - scores.max(axis=-1, keepdims=True)
    e = np.exp(scores)
    attn = e / e.sum(axis=-1, keepdims=True)
    out = np.einsum("bhts,bshd->bthd", attn, V).reshape(B, T, DQ)
    out = out @ Wo + bo
    return out.astype(np.float32), attn.astype(np.float32)


def kernel(query, key, value, key_padding_mask, attn_mask,
           Wq, bq, Wk, bk, Wv, bv, Wo, bo):
    query = np.asarray(query, np.float32)
    key = np.asarray(key, np.float32)
    value = np.asarray(value, np.float32)
    key_padding_mask = np.asarray(key_padding_mask)
    attn_mask = np.asarray(attn_mask, np.float32)
    Wq, Wk, Wv, Wo = (np.asarray(w, np.float32) for w in (Wq, Wk, Wv, Wo))
    bq, bk, bv, bo = (np.asarray(b, np.float32) for b in (bq, bk, bv, bo))

    fast = (
        not attn_mask.any()
        and np.all(key_padding_mask != 0)
        and not bq.any() and not bk.any() and not bv.any() and not bo.any()
    )
    if not fast:
        return _fallback(query, key, value, key_padding_mask, attn_mask,
                         Wq, bq, Wk, bk, Wv, bv, Wo, bo)

    if "nc" not in _prog_cache:
        _prog_cache["nc"] = _build_program()
    nc = _prog_cache["nc"]

    wq_b = (Wq / np.float32(np.sqrt(HD))).astype(BF16)
    wk_b = Wk.astype(BF16)
    wv_b = Wv.astype(BF16)

    in_maps = []
    for c in range(N_CORES):
        b, sh = c // 2, c % 2
        in_maps.append({
            "qT": np.ascontiguousarray(query[b].T).astype(BF16),
            "kT": np.ascontiguousarray(
                key[b, sh * SC:(sh + 1) * SC, :].T).astype(BF16),
            "vT": np.ascontiguousarray(
                value[b, sh * SC:(sh + 1) * SC, :].T).astype(BF16),
            "wq": wq_b, "wk": wk_b, "wv": wv_b,
        })

    res = bass_utils.run_bass_kernel_spmd(nc, in_maps, core_ids=list(range(N_CORES)))

    out_full = np.empty((B, T, DQ), np.float32)
    attn_full = np.empty((B, H, T, S), np.float32)
    for b in range(B):
        p0 = res.results[2 * b]["pv"]          # [H, 65, T] f32
        p1 = res.results[2 * b + 1]["pv"]
        inv = 1.0 / (p0[:, HD, :] + p1[:, HD, :])          # [H, T]
        for sh in range(2):
            e = res.results[2 * b + sh]["attn_e"]          # [H, SC, T] bf16
            attn_full[b, :, :, sh * SC:(sh + 1) * SC] = (
                e.astype(np.float32) * inv[:, None, :]).transpose(0, 2, 1)
        ctx = (p0[:, :HD, :] + p1[:, :HD, :]) * inv[:, None, :]  # [H, 64, T]
        out_full[b] = (
            ctx.transpose(2, 0, 1).reshape(T, DQ) @ Wo)
    return out_full, attn_full
